# revision 18
# baseline (speedup 1.0000x reference)
"""Trainium2 Bass kernel for ArcDecoder pair scoring.

Reference computation (N=768 nodes, H=128 features):
    pairs (i, j), i != j:  out[i,j] = W2 @ relu(W1a @ z_i + W1b @ z_j + b1) + b2

Device-side work is only the O(N^2 * H) part:
    H_a = relu(Bt + Abias[:, a]);  out[a, :] = W2 @ H_a
with Abias = W1a @ z.T + b1 and Bt = W1b @ z.T precomputed on host
(0.5% of the FLOPs); b2 is added host-side during assembly.

v5 (per core, 96 i-rows = 32 rounds x 3 col groups):
  - head: one fat Bt DMA on sync (1536B rows; the two HWDGE queues
    share DMA engines so column-splits are slower), AB on scalar
    behind the ACT spline prewarm; S zero-fill moved to the idle
    GPSIMD so ACT's first H tile starts as soon as Bt lands; a few
    dummy matmuls on a memset tile keep HAM warm through the DMA wait.
  - window: DVE:ACT tile split 69:27 (measured 330ns vs 826ns per
    [128,768] tile) -- the two-engine elementwise roofline.
  - matmuls: zero-padded W2 stationary (diag position q) accumulates
    row q of each 32-partition col-group strip.
  - tail: generations (28, 4); evictions cast PSUM f32 -> bf16 (halves
    the output DMA); gen-1's three strip DMAs alternate sync/scalar so
    two DGE configs run in parallel.
"""

import numpy as np
import ml_dtypes

import concourse.bass as bass
import concourse.tile as tile
from concourse import bacc, mybir
from concourse.bass_utils import run_bass_kernel_spmd

N = 768
H = 128
NCORES = 8
ROWS = N // NCORES          # 96 i-rows per core
NGRP = 3                    # PE column groups (PSUM partitions 32g..32g+31)
RND = ROWS // NGRP          # 32 rounds; round r, group g handles a = 3r + g
GRND0 = 28                  # rounds in PSUM generation 0
GRND1 = RND - GRND0         # 4 rounds in generation 1 (short tail)
QMAX = GRND0
HALF = N // 2               # 384, PSUM bank limit for f32 is 512

# rounds where all 3 tiles go to DVE (ACT does 27 tiles total);
# round 29 coincides with gen-0's eviction copies, 31 keeps the tail fast.
ALL_DVE_ROUNDS = frozenset((6, 14, 22, 29, 31))

_F32 = mybir.dt.float32
_BF16 = mybir.dt.bfloat16

_cache = {}


def _build():
    nc = bacc.Bacc(
        "TRN2",
        target_bir_lowering=False,
        debug=False,
        enable_asserts=False,
        num_devices=NCORES,
    )

    ab_d = nc.dram_tensor("AB", [H, ROWS + 1], _F32, kind="ExternalInput")
    bt_d = nc.dram_tensor("Bt", [H, N], _BF16, kind="ExternalInput")
    # each generation's rows go out as ONE padded partition-range DMA
    # (single DGE config in the tail); the host picks the valid strips
    out0_d = nc.dram_tensor("out0", [64 + GRND0, N], _BF16, kind="ExternalOutput")
    out1_d = nc.dram_tensor("out1", [64 + GRND1, N], _BF16, kind="ExternalOutput")

    relu = mybir.ActivationFunctionType.Relu
    copyf = mybir.ActivationFunctionType.Copy
    add_op = mybir.AluOpType.add
    max_op = mybir.AluOpType.max

    with tile.TileContext(nc) as tc:
        with (
            tc.tile_pool(name="const", bufs=1) as cpool,
            tc.tile_pool(name="hpool", bufs=10) as hpool,
            tc.tile_pool(name="opool", bufs=2) as opool,
            tc.tile_pool(name="psum", bufs=2, space=bass.MemorySpace.PSUM) as pspool,
        ):
            # ACT spline-table prewarm (Relu + Copy) is the first scalar-queue
            # op so the one-time ACT_TABLE_LOAD overlaps the input DMAs.
            scratch = cpool.tile([1, 8], _F32, tag="scratch")
            nc.gpsimd.memset(scratch[:], 0.0)
            nc.scalar.activation(scratch[:], scratch[:], relu)
            nc.scalar.activation(scratch[:], scratch[:], copyf)

            # fat 1536B rows on one queue beat a column-split (the two
            # HWDGE queues share the same DMA engines / bandwidth)
            bt_sb = cpool.tile([H, N], _BF16)
            nc.sync.dma_start(bt_sb[:], bt_d[:])
            ab_sb = cpool.tile([H, ROWS + 1], _F32)
            nc.scalar.dma_start(ab_sb[:], ab_d[:])
            bt = bt_sb[:]

            # PE warmup: dummy matmuls on a memset tile keep the PE busy
            # through the input-DMA wait so HAM un-throttles to 2.4 GHz.
            wtile = cpool.tile([H, HALF], _BF16, tag="wtile")
            nc.gpsimd.memset(wtile[:], 0.0)
            ps_w = pspool.tile([32, HALF], _F32, tag="psw")
            for w in range(4):
                nc.tensor.matmul(
                    ps_w[:], wtile[:, 0:32], wtile[:], start=True, stop=True
                )

            # Zero-padded W2 stationary S[k, q, q] = W2[0, k], else 0.
            # Zero-fill on the otherwise-idle GPSIMD (keeps ACT free); the
            # tiny diagonal write runs on DVE while it waits for Bt.
            S_sb = cpool.tile([H, QMAX, 32], _BF16)
            S_flat = S_sb[:].rearrange("k q m -> k (q m)")
            nc.gpsimd.memset(S_flat, 0.0)
            diag = S_flat[:, 0 : (QMAX - 1) * 33 + 1 : 33]
            nc.vector.tensor_copy(diag, ab_sb[:, ROWS : ROWS + 1].broadcast_to([H, QMAX]))

            gen_rounds = (GRND0, GRND1)
            gen_base = (0, GRND0)
            deferred_evict = []

            gen_out = (out0_d, out1_d)

            def emit_evict(t):
                grnd = gen_rounds[t]
                ps, ot = deferred_evict.pop(0)
                # PSUM f32 -> SBUF bf16 casts; ACT takes h0, DVE h1.
                nc.scalar.activation(ot[:, 0:HALF], ps[0][:], copyf)
                nc.vector.tensor_copy(ot[:, HALF:N], ps[1][:])
                nc.sync.dma_start(gen_out[t].ap()[:], ot[0 : 64 + grnd, :])

            for t in range(2):
                grnd = gen_rounds[t]
                ps = [
                    pspool.tile([ROWS, HALF], _F32, tag=f"ps{h}", name=f"ps{h}_{t}")
                    for h in range(2)
                ]
                for q in range(grnd):
                    r = gen_base[t] + q
                    hts = [None] * NGRP
                    engs = (
                        ("dve", "dve", "dve")
                        if r in ALL_DVE_ROUNDS
                        else ("dve", "dve", "act")
                    )
                    order = sorted(range(NGRP), key=lambda g: engs[g] == "dve")
                    for g in order:
                        a = NGRP * r + g
                        ht = hpool.tile([H, N], _BF16, tag="H", name=f"h{a}")
                        if engs[g] == "dve":
                            nc.vector.tensor_scalar(
                                ht[:], bt, ab_sb[:, a : a + 1], 0.0,
                                add_op, max_op,
                            )
                        else:
                            nc.scalar.activation(
                                ht[:], bt, relu,
                                bias=ab_sb[:, a : a + 1], scale=1.0,
                            )
                        hts[g] = ht
                    first = q == 0
                    last = q == grnd - 1
                    for h in range(2):
                        for g in range(NGRP):
                            nc.tensor.matmul(
                                ps[h][32 * g : 32 * g + 32, :],
                                S_sb[:, q, :],
                                hts[g][:, h * HALF : (h + 1) * HALF],
                                start=first,
                                stop=last,
                            )
                    # gen 0's eviction is emitted early in gen 1 so the
                    # copies don't displace boundary-round H tiles.
                    if t == 1 and q == 1:
                        emit_evict(0)
                ot = opool.tile([ROWS, N], _BF16, tag="ot", name=f"ot{t}")
                deferred_evict.append((ps, ot))
            emit_evict(1)

    nc.compile()
    return nc


def _get_nc():
    if "nc" not in _cache:
        _cache["nc"] = _build()
    return _cache["nc"]


def _prep_in_maps(z, W1, b1, W2, b2):
    z = np.asarray(z, np.float32)
    W1 = np.asarray(W1, np.float32)
    b1 = np.asarray(b1, np.float32)
    W2 = np.asarray(W2, np.float32)

    bf = ml_dtypes.bfloat16
    zT = z.T  # [H, N]
    abias = W1[:, :H] @ zT + b1[:, None]            # [H, N] f32
    bt = (W1[:, H:] @ zT).astype(bf)                # [H, N] bf16

    in_maps = []
    for c in range(NCORES):
        ab = np.empty((H, ROWS + 1), np.float32)
        ab[:, :ROWS] = abias[:, c * ROWS : (c + 1) * ROWS]
        ab[:, ROWS] = W2[0]
        in_maps.append({"AB": ab, "Bt": bt})
    return in_maps


def _assemble(results, b2):
    full = np.empty((N, N), np.float32)
    for c in range(NCORES):
        o0 = np.asarray(results[c]["out0"], np.float32)  # [92, 768] padded
        o1 = np.asarray(results[c]["out1"], np.float32)  # [68, 768] padded
        blk = full[c * ROWS : (c + 1) * ROWS]
        for g in range(NGRP):
            # rows a = 3q + g; gen0 q<28 at o0[32g+q], gen1 at o1[32g+q']
            blk[g : NGRP * GRND0 : NGRP] = o0[32 * g : 32 * g + GRND0]
            blk[NGRP * GRND0 + g :: NGRP] = o1[32 * g : 32 * g + GRND1]
    full += b2
    mask = ~np.eye(N, dtype=bool)
    return full[mask]  # pair-major order: i-major, j ascending, j != i


def run(z, W1, b1, W2, b2, trace=False, tmpdir=None):
    nc = _get_nc()
    in_maps = _prep_in_maps(z, W1, b1, W2, b2)
    res = run_bass_kernel_spmd(
        nc, in_maps, core_ids=list(range(NCORES)), trace=trace, tmpdir=tmpdir
    )
    return _assemble(res.results, float(np.asarray(b2, np.float32)[0])), res


def kernel(z, W1, b1, W2, b2):
    out, _ = run(z, W1, b1, W2, b2, trace=False)
    return out
